# revision 21
# baseline (speedup 1.0000x reference)
"""Trainium2 Bass kernel for nn_Loss_fn_21852793602431 (DETR-style loss).

Strategy (data-parallel over batch B=64, 8 cores x 8 batches):
  - host: preprocess per-box quantities (f32, mirroring the reference
    formula order) into per-core device inputs
  - device (SPMD x8): build the transposed DIoU pair-cost slab
    slabT[b, m, n] = dist/diag - iou_e  for the core's 8 batches, plus the
    core-partial L1 cost matrix sum_{b_loc,c} |pred - tgt| (transposed)
  - host: label (BCE) cost via matmul, sum the 8 L1 partials, add, run
    scipy linear_sum_assignment per batch, compute the final scalar losses

Device kernel layout: partitions = m (two 128-blocks), free = (b_page, n)
mega-tiles [128, 8, 256]. Four-input pairwise terms (intersection width,
enclosing-box width) use runtime-registered custom DVE ops with per-partition
target scalars, written page-by-page; two-tensor combining steps run as
whole-mega-tile ops split across DVE/GPSIMD; center-distance squares run on
the scalar engine (Square activation with per-partition bias); the L1 cost
uses |pred+(-tgt)| tiles (DVE custom / ACT Abs) contracted over (batch,coord)
by TensorE matmuls with a 0/1 selector into PSUM.
"""

import sys

if "/opt/trn_rl_repo" not in sys.path:
    sys.path.insert(0, "/opt/trn_rl_repo")

import numpy as np

B, N, M = 64, 256, 256
NCORES = 8
BL = B // NCORES
NQ = 6               # pred: x1,y1,x2,y2,cx,cy
NQT = 9              # tgt: x1,y1,x2,y2,area+eps,-cx,-cy,w,h
EPS32 = np.float32(1e-7)

CFG = {
    "nr": False,         # Newton-refine the fast reciprocals (2 ULP vs 51 ULP)
    "l1_dve_chunks": 0,  # of 8 chunks (8 m-groups each): this many on DVE
}

_bass_module = None
_custom_ops = None


def _register_custom_ops():
    global _custom_ops
    if _custom_ops is not None:
        return _custom_ops
    from concourse.dve_ops import (DveOp, OPS, CUSTOM_DVE_SPECS,
                                   _SUB_OPCODE_FOR_NAME, _CUSTOM_DVE_ROW_BASE)
    from concourse.dve_spec import (Spec, Src0, Src1, C0, C1, C2, Zero,
                                    relu, sq, maxx, minn, lower, _has_src1)
    from concourse.dve_uop import DveOpSpec

    existing = {op.name: op for op in OPS}

    def reg(name, body, reference):
        if name in existing:
            return existing[name]
        row = _CUSTOM_DVE_ROW_BASE + len(OPS)
        assert row < 0x20, "custom DVE opcode rows exhausted"
        sha = {}
        for ver in ("v3", "v4"):
            s = DveOpSpec(name=name, opcode=row,
                          uops=lower(Spec(body=body), ver=ver),
                          rd1_en=_has_src1(Spec(body=body)))
            sha[ver] = s.sha(ver)
        op = DveOp(name, Spec(body=body, reference=reference),
                   subdim=False, uops_sha=sha)
        OPS.append(op)
        _SUB_OPCODE_FOR_NAME[name] = row
        CUSTOM_DVE_SPECS[name] = op.spec
        return op

    eps = float(EPS32)
    _custom_ops = {
        # dx = min(x2p, x2t) - max(x1p, x1t)
        "DX": reg("ANT_DX", minn(Src0, C0) - maxx(Src1, C1),
                  lambda in0, in1, s0, s1, imm2:
                  np.minimum(in0, s0) - np.maximum(in1, s1)),
        # inter = relu(dx) * relu(dy)
        "MULRELU": reg("ANT_MULRELU", relu(Src0) * relu(Src1),
                       lambda in0, in1, s0, s1, imm2:
                       np.maximum(in0, 0) * np.maximum(in1, 0)),
        # ex2 = (wsum - dx)^2   [enclosing width via min+max identity]
        "SQSUB": reg("ANT_SQSUB", sq(Src0 - Src1),
                     lambda in0, in1, s0, s1, imm2: (in0 - in1) ** 2),
        # ey2e = (wsum - dy)^2 + imm2 (eps)
        "SQSUBE": reg("ANT_SQSUBE", sq(Src0 - Src1) + C2,
                      lambda in0, in1, s0, s1, imm2:
                      (in0 - in1) ** 2 + np.float32(imm2)),
        # l1 term: |S0 + S1|  (S1 carries -tgt, broadcast along n)
        "ABSADD": reg("ANT_ABSADD",
                      maxx(Src0 + Src1, Zero - (Src0 + Src1)),
                      lambda in0, in1, s0, s1, imm2: np.abs(in0 + in1)),
    }
    return _custom_ops


def _build_bass():
    import concourse.bacc as bacc
    from concourse import mybir, tile
    from contextlib import ExitStack

    ops = _register_custom_ops()
    f32 = mybir.dt.float32
    AF = mybir.ActivationFunctionType

    nc = bacc.Bacc("TRN2", target_bir_lowering=False, debug=False,
                   num_devices=NCORES)
    predq = nc.dram_tensor("predq", [NQ, BL, N], f32, kind="ExternalInput").ap()
    tgtq = nc.dram_tensor("tgtq", [2, 128, BL, NQT], f32, kind="ExternalInput").ap()
    predl1 = nc.dram_tensor("predl1", [128, 4 * N], f32, kind="ExternalInput").ap()
    tgtl1n = nc.dram_tensor("tgtl1n", [128, 64], f32, kind="ExternalInput").ap()
    selb = nc.dram_tensor("selb", [128, 256], f32, kind="ExternalInput").ap()
    slab = nc.dram_tensor("slab", [M, BL, N], f32, kind="ExternalOutput").ap()
    l1p = nc.dram_tensor("l1p", [M, N], f32, kind="ExternalOutput").ap()

    vec, gps, act = nc.vector, nc.gpsimd, nc.scalar
    flat = lambda ap: ap.rearrange("p a b -> p (a b)")

    with tile.TileContext(nc) as tc:
        with ExitStack() as ctx:
            pb = ctx.enter_context(tc.tile_pool(name="pb", bufs=1))
            tg = ctx.enter_context(tc.tile_pool(name="tg", bufs=2))
            wk = ctx.enter_context(tc.tile_pool(name="wk", bufs=2))
            ot = ctx.enter_context(tc.tile_pool(name="ot", bufs=2))
            l1w = ctx.enter_context(tc.tile_pool(name="l1w", bufs=2))
            cst = ctx.enter_context(tc.tile_pool(name="cst", bufs=1))
            psp = ctx.enter_context(tc.tile_pool(name="psp", bufs=1, space="PSUM"))

            # ---- pred broadcast tiles [128, BL, 256], one per quantity ----
            PB = []
            for q in range(NQ):
                t = pb.tile([128, BL, N], f32, tag=f"pb{q}", name=f"pb{q}")
                nsplit = 8 if q < 4 else 2
                step = BL // nsplit
                for g in range(nsplit):
                    nc.sync.dma_start(
                        t[:, g * step:(g + 1) * step, :],
                        predq[q, g * step:(g + 1) * step, :]
                        .partition_broadcast(128))
                PB.append(t)
            # derived: wp, hp, area_p (filled in halves inside mb=0)
            wp = pb.tile([128, BL, N], f32, name="wp")
            hp = pb.tile([128, BL, N], f32, name="hp")
            areap = pb.tile([128, BL, N], f32, name="areap")

            # ---- L1 cost: |pred + (-tgt)| contracted over (b, c) by PE ----
            pl1 = cst.tile([128, 4, N], f32, name="pl1")
            nc.sync.dma_start(pl1[:], predl1.rearrange("p (s n) -> p s n", s=4))
            tl1 = cst.tile([128, 64], f32, name="tl1")
            nc.sync.dma_start(tl1[:], tgtl1n)
            o4 = cst.tile([128, 256], f32, name="o4")
            nc.sync.dma_start(o4[:], selb)
            pss = [psp.tile([128, N], f32, name=f"ps{i}") for i in range(2)]

            nchunk_dve = CFG["l1_dve_chunks"]
            for chunk in range(16):          # 4 m-groups per chunk
                d = l1w.tile([128, 4, N], f32, tag="d", name="d")
                if chunk < nchunk_dve:
                    tb = tl1[:, chunk * 4:(chunk + 1) * 4]
                    vec._custom_dve(ops["ABSADD"], out=d[:],
                                    in0=pl1[:],
                                    in1=tb[:, :, None].broadcast_to((128, 4, N)))
                else:
                    for j in range(4):
                        g = chunk * 4 + j
                        act.activation(d[:, j, :], pl1[:, j, :], AF.Abs,
                                       bias=tl1[:, g:g + 1])
                for j in range(4):
                    g = chunk * 4 + j
                    mb, loc = g // 32, g % 32
                    nc.tensor.matmul(pss[mb][:],
                                     o4[:, 124 - 4 * loc:252 - 4 * loc],
                                     d[:, j, :],
                                     start=(loc == 0), stop=(loc == 31))
            for mb in range(2):
                l1sb = cst.tile([128, N], f32, tag="l1sb", name=f"l1sb{mb}")
                nc.scalar.copy(l1sb[:], pss[mb][:])
                nc.sync.dma_start(l1p[mb * 128:(mb + 1) * 128, :], l1sb[:])

            # ---- main DIoU slab: (m-block, page-chunk) iterations ----
            HLIST = [(0, 5), (5, 3)]  # (page offset, pages): small last chunk

            for mb in range(2):
                T = tg.tile([128, BL, NQT], f32, tag="T", name="T")
                nc.sync.dma_start(T[:], tgtq[mb])

                for (p0, HP) in HLIST:
                    def wt(tag):
                        return wk.tile([128, HP, N], f32, tag=tag, name=tag)

                    pg = lambda ap: ap[:, p0:p0 + HP, :]

                    def bc(qi):
                        return (T[:, p0:p0 + HP, qi][:, :, None]
                                .broadcast_to((128, HP, N)))

                    dx = wt("dx"); dy = wt("dy")
                    dcx = wt("dcx"); dcy = wt("dcy")
                    for bb in range(HP):
                        b = p0 + bb
                        s = lambda qi: T[:, b, qi:qi + 1]
                        vec._custom_dve(ops["DX"], out=dx[:, bb, :],
                                        in0=PB[2][:, b, :], in1=PB[0][:, b, :],
                                        s0=s(2), s1=s(0))
                        vec._custom_dve(ops["DX"], out=dy[:, bb, :],
                                        in0=PB[3][:, b, :], in1=PB[1][:, b, :],
                                        s0=s(3), s1=s(1))
                        # (cx_p + (-cx_t))^2
                        act.activation(dcx[:, bb, :], PB[4][:, b, :],
                                       AF.Square, bias=s(5))
                        act.activation(dcy[:, bb, :], PB[5][:, b, :],
                                       AF.Square, bias=s(6))

                    if mb == 0:
                        gps.tensor_sub(pg(wp[:]), pg(PB[2][:]), pg(PB[0][:]))
                        gps.tensor_sub(pg(hp[:]), pg(PB[3][:]), pg(PB[1][:]))
                        vec.tensor_mul(pg(areap[:]), pg(wp[:]), pg(hp[:]))
                    inter = wt("inter")
                    vec._custom_dve(ops["MULRELU"], out=flat(inter[:]),
                                    in0=flat(dx[:]), in1=flat(dy[:]))
                    u1 = wt("u1")
                    gps.tensor_add(u1[:], pg(areap[:]), bc(4))
                    gps.tensor_sub(u1[:], u1[:], inter[:])       # une
                    ex2 = wt("ex2"); ey2e = wt("ey2e")
                    gps.tensor_add(ex2[:], pg(wp[:]), bc(7))     # wp + wt
                    gps.tensor_add(ey2e[:], pg(hp[:]), bc(8))    # hp + ht
                    vec._custom_dve(ops["SQSUB"], out=flat(ex2[:]),
                                    in0=flat(ex2[:]), in1=flat(dx[:]))
                    vec._custom_dve(ops["SQSUBE"], out=flat(ey2e[:]),
                                    in0=flat(ey2e[:]), in1=flat(dy[:]),
                                    imm2=float(EPS32))
                    gps.tensor_add(ex2[:], ex2[:], ey2e[:])      # diag_e
                    vec.tensor_add(dcx[:], dcx[:], dcy[:])       # dist

                    rfu = wt("rfu"); rfd = wt("rfd")
                    vec.reciprocal_approx_fast(rfu[:], u1[:])
                    vec.reciprocal_approx_fast(rfd[:], ex2[:])
                    if CFG["nr"]:
                        from concourse.dve_ops import RECIPROCAL_APPROX_NR
                        vec._custom_dve(RECIPROCAL_APPROX_NR, out=flat(rfu[:]),
                                        in0=flat(u1[:]), in1=flat(rfu[:]),
                                        s0=2.0)
                        vec._custom_dve(RECIPROCAL_APPROX_NR, out=flat(rfd[:]),
                                        in0=flat(ex2[:]), in1=flat(rfd[:]),
                                        s0=2.0)
                    gps.tensor_mul(inter[:], inter[:], rfu[:])   # q = iou_e
                    vec.tensor_mul(dcx[:], dcx[:], rfd[:])       # dd
                    outm = ot.tile([128, HP, N], f32, tag="outm", name="outm")
                    vec.tensor_sub(outm[:], dcx[:], inter[:])
                    nc.sync.dma_start(
                        slab[mb * 128:(mb + 1) * 128,
                             p0:p0 + HP, :], outm[:])

    nc.compile()
    return nc


def _get_bass():
    global _bass_module
    if _bass_module is None:
        _bass_module = _build_bass()
    return _bass_module


def _preprocess(bbox_pred, bbox_target):
    """Host-side per-box quantities, f32 ops mirroring the reference."""
    f32 = np.float32
    bp = np.asarray(bbox_pred, dtype=f32)
    bt = np.asarray(bbox_target, dtype=f32)
    cx, cy, w, h = bp[..., 0], bp[..., 1], bp[..., 2], bp[..., 3]
    px1 = cx - w / 2; px2 = cx + w / 2
    py1 = cy - h / 2; py2 = cy + h / 2
    parea = (px2 - px1) * (py2 - py1)
    psx = px1 + px2; psy = py1 + py2
    predq = np.stack([px1, py1, px2, py2, cx, cy],
                     axis=0).astype(f32)                       # [NQ, B, N]
    gx, gy, gw, gh = bt[..., 0], bt[..., 1], bt[..., 2], bt[..., 3]
    tx1 = gx - gw / 2; tx2 = gx + gw / 2
    ty1 = gy - gh / 2; ty2 = gy + gh / 2
    tarea_eps = (tx2 - tx1) * (ty2 - ty1) + EPS32
    tgtq = np.stack([tx1, ty1, tx2, ty2, tarea_eps, -gx, -gy, gw, gh],
                    axis=2).astype(f32)                        # [B, M, NQT]
    # device layout: [2(mb), 128(m), BL, NQT] per core
    tgtq = np.ascontiguousarray(
        tgtq.reshape(NCORES, BL, 2, 128, 9).transpose(0, 2, 3, 1, 4))

    # L1 inputs per core: partition j = rep*32 + b_loc*4 + c
    rep = np.arange(128) // 32
    bj = (np.arange(128) % 32) // 4
    cj = np.arange(128) % 4
    predl1 = np.empty((NCORES, 128, N), dtype=f32)  # replicated x8 below
    tgtl1n = np.empty((NCORES, 128, 64), dtype=f32)
    g = np.arange(64)
    mm = 4 * g[None, :] + rep[:, None]                         # [128, 64]
    for core in range(NCORES):
        bg = core * BL + bj
        predl1[core] = bp[bg, :, cj]
        tgtl1n[core] = -bt[bg[:, None], mm, cj[:, None]]
    selb = (np.arange(256)[None, :] == 124 + rep[:, None]).astype(f32)
    predl1r = np.ascontiguousarray(
        np.broadcast_to(predl1[:, :, None, :], (NCORES, 128, 4, N))
    ).reshape(NCORES, 128, 4 * N)
    return predq, tgtq, predl1r, tgtl1n, selb


def _label_cost_T(labels_pred, labels_target):
    """lcT[m, n] = mean_b bce(p[b,n], t[b,m]); f32 elementwise like jax."""
    f32 = np.float32
    x = np.asarray(labels_pred, dtype=f32)[..., 0]
    p = (f32(1.0) / (f32(1.0) + np.exp(-x))).astype(f32)
    lnp = np.maximum(np.log(p), f32(-100.0)).astype(f32)
    ln1 = np.maximum(np.log((f32(1.0) - p).astype(f32)), f32(-100.0)).astype(f32)
    t = np.asarray(labels_target, dtype=np.float64)            # [B, M]
    a = lnp.astype(np.float64); c = ln1.astype(np.float64)     # [B, N]
    return -(t.T @ a + (1.0 - t.T) @ c) / B                    # [M, N] f64


def _solve_assignments(costT):
    """costT: [B, M, N] f64. Returns cols[b, n] = matched target index."""
    from scipy.optimize import linear_sum_assignment
    cols = np.empty((B, N), dtype=np.int64)

    def solve(b):
        row_ind, col_ind = linear_sum_assignment(costT[b])
        cols[b, col_ind] = row_ind

    try:
        from concurrent.futures import ThreadPoolExecutor
        with ThreadPoolExecutor(max_workers=8) as tp:
            list(tp.map(solve, range(B)))
    except Exception:
        for b in range(B):
            solve(b)
    return cols


def _final_losses(labels_pred, bbox_pred, labels_target, bbox_target, cols):
    f64 = np.float64
    bp = np.asarray(bbox_pred, dtype=f64)
    bt = np.asarray(bbox_target, dtype=f64)
    lt = np.asarray(labels_target, dtype=f64)
    x = np.asarray(labels_pred, dtype=np.float32)[..., 0]
    p32 = (np.float32(1.0) / (np.float32(1.0) + np.exp(-x))).astype(np.float32)
    p = p32.astype(f64)

    bi = np.arange(B)[:, None]
    t_m = lt[bi, cols]
    bt_m = bt[bi, cols]
    wm = (t_m == 1.0).astype(f64)

    def xyxy(bb):
        c_x, c_y, ww, hh = bb[..., 0], bb[..., 1], bb[..., 2], bb[..., 3]
        return (c_x - ww / 2, c_y - hh / 2, c_x + ww / 2, c_y + hh / 2)

    x1, y1, x2, y2 = xyxy(bp)
    xg1, yg1, xg2, yg2 = xyxy(bt_m)
    xi1 = np.maximum(x1, xg1); yi1 = np.maximum(y1, yg1)
    xi2 = np.minimum(x2, xg2); yi2 = np.minimum(y2, yg2)
    inter = np.clip(xi2 - xi1, 0, None) * np.clip(yi2 - yi1, 0, None)
    union = (x2 - x1) * (y2 - y1) + (xg2 - xg1) * (yg2 - yg1) - inter
    iou_p = inter / union
    iou_e = inter / (union + 1e-7)
    xc1 = np.minimum(x1, xg1); yc1 = np.minimum(y1, yg1)
    xc2 = np.maximum(x2, xg2); yc2 = np.maximum(y2, yg2)
    diag = (xc2 - xc1) ** 2 + (yc2 - yc1) ** 2 + 1e-7
    dist = ((x1 + x2 - xg1 - xg2) * 0.5) ** 2 + ((y1 + y2 - yg1 - yg2) * 0.5) ** 2
    diou_e = 1.0 - iou_e + dist / diag

    wsum = wm.sum()
    diou_loss = (diou_e * wm).sum() / wsum
    iou_out = (iou_p * wm).sum() / wsum
    lnp = np.maximum(np.log(p), -100.0)
    ln1 = np.maximum(np.log1p(-p), -100.0)
    label_loss = (-(t_m * lnp + (1.0 - t_m) * ln1)).mean()
    bbox_loss = (np.abs(bp - bt_m) * wm[..., None]).sum() / (wsum * 4.0)
    return diou_loss + label_loss + bbox_loss, iou_out


def kernel(labels_pred, bbox_pred, labels_target, bbox_target):
    from concourse import bass_utils

    nc = _get_bass()
    predq, tgtq, predl1, tgtl1n, selb = _preprocess(bbox_pred, bbox_target)

    in_maps = [
        {"predq": np.ascontiguousarray(predq[:, c * BL:(c + 1) * BL]),
         "tgtq": tgtq[c],
         "predl1": np.ascontiguousarray(predl1[c]),
         "tgtl1n": np.ascontiguousarray(tgtl1n[c]),
         "selb": selb}
        for c in range(NCORES)
    ]
    res = bass_utils.run_bass_kernel_spmd(nc, in_maps, core_ids=list(range(NCORES)))

    slabT = np.concatenate(
        [res.results[c]["slab"].transpose(1, 0, 2) for c in range(NCORES)],
        axis=0).astype(np.float64)                             # [B, M, N]
    l1T = sum(res.results[c]["l1p"].astype(np.float64)
              for c in range(NCORES)) / (B * 4.0)              # [M, N]
    lcT = _label_cost_T(labels_pred, labels_target)            # [M, N]

    costT = slabT + (l1T + lcT + 1.0)[None, :, :]
    cols = _solve_assignments(costT)

    total, iou = _final_losses(labels_pred, bbox_pred, labels_target,
                               bbox_target, cols)
    return np.float32(total), np.float32(iou)
